# revision 7
# baseline (speedup 1.0000x reference)
"""GCN (3-layer, symmetric-normalized, mean-pooled) on 8 Trainium2 NeuronCores.

Strategy (v3, fp8 + host-built S):
- Factor the GCN normalization: w[e] = dis[row]*dis[col] with dis = deg^-1/2.
  propagate(h) = dis ⊙ (A @ (dis ⊙ h)), so per-edge weights disappear;
  only per-node scales remain (fused into elementwise passes).
- Shard destination nodes (and their in-edges) across the 8 cores.
- Messages are fp8e4m3: the scatter-add averages ~17 edges per node, so the
  extra quantization noise stays ~2e-3 final rel err (vs 2e-2 budget).
- Gather h[col] per edge with 128-byte SWDGE descriptors (one fp8 node row)
  out of 256B-stride pair rows; idx = pair id fits int16. 128B descs halve
  the DMA cost vs bf16. Calls are capped at 1024 idx (64 idx-table columns,
  a hard Q7 limit); a 64KB descriptor carveout keeps ~4 calls in flight per
  queue so call latency (DGE+DMA+sem ~5us) pipelines away.
- Scatter-add via TensorE matmuls against one-hot S matrices PRECOMPUTED ON
  HOST and streamed from HBM per (block, parity) segment with big regular
  DMAs (frees the Vector engine). fp8 DoubleRow packs 2 chunks (256 edges)
  per matmul, halving PE instruction count.
- Tile-block-major processing: each block of 7 dest tiles issues its gather
  calls (both parities), then accumulates each tile's chunks in ONE psum
  pass (no zA bounce buffer).
- AllGather of the fp8 shards between layers; global mean-pool interleaved
  into layer 3's per-tile epilogue via one-hot matmuls against batch ids.
"""

import numpy as np


def _ceil_div(a, b):
    return (a + b - 1) // b


class _Sched:
    pass


def _preprocess(x, edge_index, batch, n_cores=8):
    """Build the static schedule + per-core tables from the graph indices."""
    N, D = x.shape
    assert D == 128
    assert N % n_cores == 0
    s = _Sched()
    s.N, s.D, s.n_cores = N, D, n_cores
    s.shard = N // n_cores
    s.tiles = _ceil_div(s.shard, 128)
    s.shard_pad = s.tiles * 128
    s.npad = s.shard_pad * n_cores
    s.BT = 7
    s.nblocks = _ceil_div(s.tiles, s.BT)

    row = np.concatenate([np.asarray(edge_index[0]), np.arange(N, dtype=np.int64)])
    col = np.concatenate([np.asarray(edge_index[1]), np.arange(N, dtype=np.int64)])
    deg = np.bincount(row, minlength=N).astype(np.float32)
    dis = deg ** -0.5
    s.dis = dis

    # padded global index (each core's shard padded to shard_pad rows)
    colp = (col // s.shard) * s.shard_pad + (col % s.shard)

    core_of = row // s.shard
    tile_of = (row % s.shard) // 128
    parity = colp & 1

    # order edges by (core, block, parity, tile)
    block_of = tile_of // s.BT
    key = ((core_of * s.nblocks + block_of) * 2 + parity) * s.tiles + tile_of
    order = np.argsort(key, kind="stable")
    row_s = row[order]
    colp_s = colp[order]
    core_s = core_of[order]
    tile_s = tile_of[order]
    par_s = parity[order]
    key_sorted = key[order]

    # per (core, tile, parity) counts; chunk counts = max over cores
    cnt_key = (core_s * s.tiles + tile_s) * 2 + par_s
    nkeys = n_cores * s.tiles * 2
    counts = np.bincount(cnt_key, minlength=nkeys).reshape(n_cores, s.tiles, 2)
    nch = _ceil_div(counts, 128).max(axis=0)  # [tiles, 2]
    s.nch = nch

    # chunk numbering: for block b: for parity p: for tile t in block
    choff = np.zeros((s.tiles, 2), dtype=np.int64)
    seg = []  # per (block, parity): (chunk_base, nchunks)
    a = 0
    for b in range(s.nblocks):
        t0, t1 = b * s.BT, min((b + 1) * s.BT, s.tiles)
        for p in range(2):
            base = a
            for t in range(t0, t1):
                choff[t, p] = a
                a += nch[t, p]
            seg.append((base, a - base))
    s.choff = choff
    s.totch = int(a)
    s.seg = seg
    s.maxseg = max(n for _, n in seg)

    # calls: per (block, parity) segment, balanced groups of <= G chunks
    G = 8
    s.G = G
    s.calls = []  # (chunk_base, nchunks, parity, block)
    chunk_call = np.zeros(s.totch, dtype=np.int64)
    chunk_col = np.zeros(s.totch, dtype=np.int64)
    s.calls_of_block = [[] for _ in range(s.nblocks)]
    si = 0
    for b in range(s.nblocks):
        for p in range(2):
            base, n = seg[si]
            si += 1
            if n == 0:
                continue
            ncalls = _ceil_div(n, G)
            sizes = [n // ncalls + (1 if i < n % ncalls else 0) for i in range(ncalls)]
            off = 0
            for sz in sizes:
                ci = len(s.calls)
                s.calls.append((base + off, sz, p, b))
                s.calls_of_block[b].append(ci)
                chunk_call[base + off:base + off + sz] = ci
                chunk_col[base + off:base + off + sz] = np.arange(sz)
                off += sz
    s.chunk_call = chunk_call
    s.chunk_col = chunk_col
    s.maxG = max(c[1] for c in s.calls)

    # per-core tables
    s.idx_tab = np.zeros((n_cores, 128, 8 * s.totch), dtype=np.int16)
    s.ld_tab = np.full((n_cores, 128, s.totch), -1.0, dtype=np.float32)
    for c in range(n_cores):
        for t in range(s.tiles):
            b = t // s.BT
            for p in range(2):
                n = int(nch[t, p])
                if n == 0:
                    continue
                k = ((c * s.nblocks + b) * 2 + p) * s.tiles + t
                lo = np.searchsorted(key_sorted, k, side="left")
                hi = np.searchsorted(key_sorted, k, side="right")
                cnt = hi - lo
                idx = np.zeros(n * 128, dtype=np.int64)
                idx[:cnt] = colp_s[lo:hi] >> 1
                ld = np.full(n * 128, -1.0, dtype=np.float32)
                ld[:cnt] = (row_s[lo:hi] - c * s.shard - t * 128).astype(np.float32)
                co = int(choff[t, p])
                # idx j -> [j%16, j//16], replicated across the 8 Q7 core groups
                wrapped = idx.astype(np.int16).reshape(-1, 16).T  # [16, n*8]
                s.idx_tab[c, :, 8 * co:8 * (co + n)] = np.tile(wrapped, (8, 1))
                s.ld_tab[c, :, co:co + n] = ld.reshape(n, 128).T

    # per-core dis table (partition = node % 128, col = tile), pad rows -> 0
    s.dis_t = np.zeros((n_cores, 128, s.tiles), dtype=np.float32)
    for c in range(n_cores):
        d = np.zeros(s.shard_pad, dtype=np.float32)
        d[:s.shard] = dis[c * s.shard:(c + 1) * s.shard]
        s.dis_t[c] = d.reshape(s.tiles, 128).T

    # pooling windows: split tiles into nw contiguous groups such that each
    # group's batch-id span is < 128 for every core
    batch = np.asarray(batch)
    s.B = int(batch.max()) + 1 if batch.size else 1
    for nw in range(1, s.tiles + 1):
        bounds = [round(i * s.tiles / nw) for i in range(nw + 1)]
        ok = True
        win_start = np.zeros((n_cores, nw), dtype=np.int64)
        for c in range(n_cores):
            for w in range(nw):
                n0 = c * s.shard + bounds[w] * 128
                n1 = min(c * s.shard + bounds[w + 1] * 128, (c + 1) * s.shard) - 1
                if n0 > n1:
                    win_start[c, w] = 0
                    continue
                b0, b1 = int(batch[n0]), int(batch[n1])
                if b1 - b0 > 127:
                    ok = False
                    break
                win_start[c, w] = b0
            if not ok:
                break
        if ok:
            s.nw = nw
            s.wbounds = bounds
            s.win_start = win_start
            break
    else:
        raise RuntimeError("no pooling window split found")
    s.win_of_tile = np.zeros(s.tiles, dtype=np.int64)
    for w in range(s.nw):
        s.win_of_tile[s.wbounds[w]:s.wbounds[w + 1]] = w

    # local graph ids per (core, tile): batch[node] - win_start, pad -> -1
    s.lg_tab = np.full((n_cores, 128, s.tiles), -1.0, dtype=np.float32)
    for c in range(n_cores):
        lg = np.full(s.shard_pad, -1.0, dtype=np.float32)
        bshard = batch[c * s.shard:(c + 1) * s.shard].astype(np.float32)
        for w in range(s.nw):
            t0, t1 = s.wbounds[w], s.wbounds[w + 1]
            n0, n1 = t0 * 128, min(t1 * 128, s.shard)
            lg[n0:n1] = bshard[n0:n1] - s.win_start[c, w]
        s.lg_tab[c] = lg.reshape(s.tiles, 128).T

    s.cnts = np.bincount(batch, minlength=s.B).astype(np.float32)
    return s


def _raw_gather(gp, out_ap, in_ap, idxs_ap, num_idxs, elem_size, elem_step,
                queue_num):
    """dma_gather without the elem_size_bytes%256 wrapper assert.

    The ucode only requires the source row STRIDE to be a multiple of 256B
    (stride_bytes_256 field); the per-descriptor payload is elem_size bytes.
    128B fp8 payloads at 256B stride are legal and halve the DMA cost.
    """
    from concourse import mybir

    gp._assert_queue_num(queue_num)
    assert idxs_ap.dtype == mybir.dt.int16
    dts = mybir.dt.size(in_ap.dtype)
    stride_bytes = elem_step * dts
    assert stride_bytes % 256 == 0 and stride_bytes // 256 < 256
    assert in_ap.ap[0][0] == elem_step
    assert in_ap.ap[-1][1] == out_ap.ap[-1][1] == elem_size
    _in_ap = gp.lower_ap_dma(in_ap, for_custom_bir_dma=True)
    _idxs_ap = gp.lower_ap(idxs_ap)
    _out_ap = gp.lower_ap(out_ap)
    return gp.add_instruction(
        mybir.InstDMAGatherAnt(
            name=gp.bass.get_next_instruction_name(),
            ins=[*_in_ap, _idxs_ap, gp.lower_val_access(gp.to_reg(num_idxs))],
            outs=[_out_ap],
            transpose=False,
            num_idxs=num_idxs,
            elem_size=elem_size,
            stride_bytes_256=stride_bytes // 256,
            gen_mode=0,
            single_packet=True,
            queue_num=queue_num,
            sbuf_tokens_per_rank=0,
            sbuf_free_dim_per_rank=0,
            sbuf_free_dim_pad_per_rank=0,
            sbuf_byte_offset=0,
        )
    )


def _build(s, layers=3):
    """Build the shared SPMD Bass/Tile program."""
    from contextlib import ExitStack

    import concourse.tile as tile
    from concourse import bacc, mybir

    DT = mybir.dt
    F32 = DT.float32
    BF16 = DT.bfloat16
    FP8 = DT.float8e4
    nc = bacc.Bacc("TRN2", target_bir_lowering=False, debug=False,
                   num_devices=s.n_cores, num_swdge_queues=4,
                   dynamic_dma_scratch_size=32768)

    g0 = nc.dram_tensor("g0", [s.npad, 128], FP8, kind="ExternalInput")
    w_in = nc.dram_tensor("w_in", [layers, 128, 128], BF16, kind="ExternalInput")
    b_in = nc.dram_tensor("b_in", [layers, 128, 128], F32, kind="ExternalInput")
    iota_in = nc.dram_tensor("iota_in", [128, 128], BF16, kind="ExternalInput")
    idx_in = nc.dram_tensor("idx_in", [128, 8 * s.totch], DT.int16, kind="ExternalInput")
    s_in = nc.dram_tensor("s_in", [128, s.totch * 128], FP8, kind="ExternalInput")
    dis_in = nc.dram_tensor("dis_in", [128, s.tiles], F32, kind="ExternalInput")
    lg_in = nc.dram_tensor("lg_in", [128, s.tiles], F32, kind="ExternalInput")
    pool_out = nc.dram_tensor("pool_out", [s.nw, 128, 128], F32, kind="ExternalOutput")

    g_bounce = nc.dram_tensor("g_bounce", [s.shard_pad, 128], FP8)
    g_full = [
        nc.dram_tensor(f"g_full{l}", [s.npad, 128], FP8, addr_space="Shared")
        for l in range(1, layers)
    ]

    relu = mybir.ActivationFunctionType.Relu
    copyf = mybir.ActivationFunctionType.Copy
    iseq = mybir.AluOpType.is_equal
    mult = mybir.AluOpType.mult
    add = mybir.AluOpType.add
    dr = mybir.MatmulPerfMode.DoubleRow

    with tile.TileContext(nc) as tc, ExitStack() as ctx:
        const = ctx.enter_context(tc.tile_pool(name="const", bufs=1))
        msgp = ctx.enter_context(tc.tile_pool(name="msg", bufs=48))
        sp = ctx.enter_context(tc.tile_pool(name="sp", bufs=5))
        pp = ctx.enter_context(tc.tile_pool(name="pp", bufs=4))
        zp = ctx.enter_context(tc.tile_pool(name="zp", bufs=8))
        houtp = ctx.enter_context(tc.tile_pool(name="hout", bufs=1))
        psz = ctx.enter_context(tc.tile_pool(name="psz", bufs=3, space="PSUM"))
        psu = ctx.enter_context(tc.tile_pool(name="psu", bufs=2, space="PSUM"))
        pspool = ctx.enter_context(tc.tile_pool(name="pspool", bufs=2, space="PSUM"))

        iota_t = const.tile([128, 128], BF16)
        nc.sync.dma_start(iota_t[:], iota_in[:])
        idx_t = const.tile([128, 8 * s.totch], DT.int16)
        nc.sync.dma_start(idx_t[:], idx_in[:])
        dis_t = const.tile([128, s.tiles], F32)
        nc.sync.dma_start(dis_t[:], dis_in[:])
        lg_t = const.tile([128, s.tiles], F32)
        nc.sync.dma_start(lg_t[:], lg_in[:])
        w_t = const.tile([128, layers, 128], BF16)
        nc.sync.dma_start(w_t[:], w_in.ap().rearrange("l p d -> p l d"))
        b_t = const.tile([128, layers, 128], F32)
        nc.sync.dma_start(b_t[:], b_in.ap().rearrange("l p d -> p l d"))

        hout = houtp.tile([128, s.tiles * 128], FP8)
        h3 = houtp.tile([128, s.tiles * 128], BF16)

        pool_pz = None

        for l in range(layers):
            g_src = g0 if l == 0 else g_full[l - 1]
            g_pair = g_src.ap().rearrange("(n two) d -> n (two d)", two=2)

            if l == layers - 1:
                pool_pz = [pspool.tile([128, 128], F32, tag="pool",
                                       name=f"poolpz{w}")
                           for w in range(s.nw)]

            call_tiles = {}
            seg_tiles = {}
            qn = 0

            for b in range(s.nblocks):
                # stream this block's S segments (host-precomputed one-hots)
                for p in range(2):
                    sbase, slen = s.seg[b * 2 + p]
                    if slen == 0:
                        continue
                    st = sp.tile([128, s.maxseg * 128], FP8, tag="S")
                    nc.sync.dma_start(
                        st[:, 0:slen * 128],
                        s_in[:, sbase * 128:(sbase + slen) * 128],
                    )
                    seg_tiles[(b, p)] = (st, sbase)
                # issue this block's gather calls
                for ci in s.calls_of_block[b]:
                    base, n, p, _ = s.calls[ci]
                    m = msgp.tile([128, s.maxG, 128], FP8, tag="msg")
                    _raw_gather(
                        nc.gpsimd,
                        m[:, 0:n, :],
                        g_pair[:, p * 128:(p + 1) * 128],
                        idx_t[:, 8 * base:8 * (base + n)],
                        n * 128,
                        128,
                        256,
                        qn,
                    )
                    qn = (qn + 1) % 4
                    call_tiles[ci] = m

                # process this block's tiles: one psum pass per tile
                t0, t1 = b * s.BT, min((b + 1) * s.BT, s.tiles)
                for t in range(t0, t1):
                    # gather (call, col, chunk, span) groups: DoubleRow pairs
                    groups = []
                    for p in range(2):
                        n = int(s.nch[t, p])
                        k = 0
                        while k < n:
                            gc = int(s.choff[t, p]) + k
                            ci = int(s.chunk_call[gc])
                            j = int(s.chunk_col[gc])
                            if k + 1 < n and int(s.chunk_call[gc + 1]) == ci:
                                groups.append((p, ci, j, gc, 2))
                                k += 2
                            else:
                                groups.append((p, ci, j, gc, 1))
                                k += 1
                    pz = psz.tile([128, 128], F32, tag="pz")
                    for i, (p, ci, j, gc, span) in enumerate(groups):
                        m = call_tiles[ci]
                        st, sbase = seg_tiles[(b, p)]
                        o = (gc - sbase) * 128
                        s_ap = st[:, o:o + span * 128] \
                            .rearrange("q (k d) -> q k d", k=span)
                        nc.tensor.matmul(
                            pz[:], m[:, j:j + span, :], s_ap,
                            start=(i == 0), stop=(i == len(groups) - 1),
                            perf_mode=(dr if span == 2 else None),
                        )
                    # epilogue: zT = copy(pz); pu = zT @ W; hout = relu(...)
                    zT = zp.tile([128, 128], BF16, tag="zT")
                    nc.scalar.activation(zT[:], pz[:], copyf)
                    pu = psu.tile([128, 128], F32, tag="pu")
                    nc.tensor.matmul(pu[:], zT[:], w_t[:, l, :], start=True, stop=True)
                    t1v = zp.tile([128, 128], F32, tag="t1")
                    nc.vector.tensor_scalar(t1v[:], pu[:], dis_t[:, t:t + 1], None, mult)
                    t2v = zp.tile([128, 128], F32, tag="t2")
                    nc.vector.tensor_tensor(t2v[:], t1v[:], b_t[:, l, :], add)
                    if l < layers - 1:
                        nc.scalar.activation(
                            hout[:, t * 128:(t + 1) * 128], t2v[:], relu,
                            scale=dis_t[:, t:t + 1],
                        )
                    else:
                        h3s = h3[:, t * 128:(t + 1) * 128]
                        nc.scalar.activation(h3s, t2v[:], relu)
                        # interleaved mean-pool accumulation
                        w = int(s.win_of_tile[t])
                        P = pp.tile([128, 128], BF16, tag="P")
                        nc.vector.tensor_scalar(
                            P[:], iota_t[:], lg_t[:, t:t + 1], None, iseq
                        )
                        nc.tensor.matmul(
                            pool_pz[w][:], P[:], h3s,
                            start=(t == s.wbounds[w]),
                            stop=(t == s.wbounds[w + 1] - 1),
                        )

            if l < layers - 1:
                nc.sync.dma_start(
                    g_bounce.ap().rearrange("(t p) d -> p t d", p=128),
                    hout[:],
                )
                nc.gpsimd.collective_compute(
                    "AllGather",
                    mybir.AluOpType.bypass,
                    replica_groups=[list(range(s.n_cores))],
                    ins=[g_bounce.ap().opt()],
                    outs=[g_full[l].ap().opt()],
                )

        for w in range(s.nw):
            pres = zp.tile([128, 128], F32, tag="pres")
            nc.vector.tensor_copy(pres[:], pool_pz[w][:])
            nc.sync.dma_start(pool_out[w], pres[:])

    nc.compile()
    return nc


def _in_maps(x, Ws, bs, s):
    import ml_dtypes

    fp8 = ml_dtypes.float8_e4m3
    bf16 = ml_dtypes.bfloat16
    g0 = np.zeros((s.npad, 128), dtype=fp8)
    gx = s.dis[:, None] * np.asarray(x, dtype=np.float32)
    for c in range(s.n_cores):
        g0[c * s.shard_pad:c * s.shard_pad + s.shard] = \
            gx[c * s.shard:(c + 1) * s.shard].astype(fp8)

    iota = np.tile(np.arange(128, dtype=np.float32), (128, 1)).astype(bf16)
    wcat = np.stack([np.asarray(w, np.float32) for w in Ws]).astype(bf16)
    bcat = np.stack([np.tile(np.asarray(b, np.float32), (128, 1)) for b in bs])
    ar = np.arange(128, dtype=np.float32)

    in_maps = []
    for c in range(s.n_cores):
        S = (s.ld_tab[c][:, :, None] == ar).astype(fp8) \
            .reshape(128, s.totch * 128)
        in_maps.append({
            "g0": g0,
            "w_in": wcat,
            "b_in": bcat,
            "iota_in": iota,
            "idx_in": s.idx_tab[c],
            "s_in": S,
            "dis_in": s.dis_t[c],
            "lg_in": s.lg_tab[c],
        })

    return in_maps


def _run(x, edge_index, batch, Ws, bs, s, nc):
    from concourse.bass_utils import run_bass_kernel_spmd

    in_maps = _in_maps(x, Ws, bs, s)
    br = run_bass_kernel_spmd(nc, in_maps, list(range(s.n_cores)))

    acc = np.zeros((s.B + 128, 128), dtype=np.float32)
    for c in range(s.n_cores):
        po = np.asarray(br.results[c]["pool_out"]).reshape(s.nw, 128, 128)
        for w in range(s.nw):
            ws = int(s.win_start[c, w])
            acc[ws:ws + 128] += po[w]
    out = acc[:s.B] / np.maximum(s.cnts, 1.0)[:, None]
    return out, br


def kernel(x, edge_index, batch, W0, b0, W1, b1, W2, b2):
    x = np.asarray(x)
    edge_index = np.asarray(edge_index)
    batch = np.asarray(batch)
    s = _preprocess(x, edge_index, batch)
    nc = _build(s)
    out, _ = _run(x, edge_index, batch, [W0, W1, W2], [b0, b1, b2], s, nc)
    return out.astype(np.float32)


# revision 10
# speedup vs baseline: 1.0664x; 1.0664x over previous
"""GCN (3-layer, symmetric-normalized, mean-pooled) on 8 Trainium2 NeuronCores.

Strategy (v3, fp8 + host-built S):
- Factor the GCN normalization: w[e] = dis[row]*dis[col] with dis = deg^-1/2.
  propagate(h) = dis ⊙ (A @ (dis ⊙ h)), so per-edge weights disappear;
  only per-node scales remain (fused into elementwise passes).
- Shard destination nodes (and their in-edges) across the 8 cores.
- Messages are fp8e4m3: the scatter-add averages ~17 edges per node, so the
  extra quantization noise stays ~2e-3 final rel err (vs 2e-2 budget).
- Gather h[col] per edge with 128-byte SWDGE descriptors (one fp8 node row)
  out of 256B-stride pair rows; idx = pair id fits int16. 128B descs halve
  the DMA cost vs bf16. Calls are capped at 1024 idx (64 idx-table columns,
  a hard Q7 limit); a 64KB descriptor carveout keeps ~4 calls in flight per
  queue so call latency (DGE+DMA+sem ~5us) pipelines away.
- Scatter-add via TensorE matmuls against one-hot S matrices PRECOMPUTED ON
  HOST and streamed from HBM per (block, parity) segment with big regular
  DMAs (frees the Vector engine). fp8 DoubleRow packs 2 chunks (256 edges)
  per matmul, halving PE instruction count.
- Tile-block-major processing: each block of 7 dest tiles issues its gather
  calls (both parities), then accumulates each tile's chunks in ONE psum
  pass (no zA bounce buffer).
- AllGather of the fp8 shards between layers; global mean-pool interleaved
  into layer 3's per-tile epilogue via one-hot matmuls against batch ids.
"""

import numpy as np


def _ceil_div(a, b):
    return (a + b - 1) // b


class _Sched:
    pass


def _preprocess(x, edge_index, batch, n_cores=8):
    """Build the static schedule + per-core tables from the graph indices."""
    N, D = x.shape
    assert D == 128
    assert N % n_cores == 0
    s = _Sched()
    s.N, s.D, s.n_cores = N, D, n_cores
    s.shard = N // n_cores
    s.tiles = _ceil_div(s.shard, 128)
    s.shard_pad = s.tiles * 128
    s.npad = s.shard_pad * n_cores
    s.BT = 7
    s.nblocks = _ceil_div(s.tiles, s.BT)

    row = np.concatenate([np.asarray(edge_index[0]), np.arange(N, dtype=np.int64)])
    col = np.concatenate([np.asarray(edge_index[1]), np.arange(N, dtype=np.int64)])
    deg = np.bincount(row, minlength=N).astype(np.float32)
    dis = deg ** -0.5
    s.dis = dis

    # padded global index, half-major: [A: per-core rows 0..rowsA) ...]
    # [B: per-core rows rowsA..shard_pad)] so the between-layer AllGather can
    # run as two contiguous-range collectives (half A overlaps layer tail).
    s.tA = (s.tiles + 1) // 2
    s.rowsA = s.tA * 128
    s.rowsB = s.shard_pad - s.rowsA
    cc = col // s.shard
    rr = col % s.shard
    colp = np.where(rr < s.rowsA,
                    cc * s.rowsA + rr,
                    n_cores * s.rowsA + cc * s.rowsB + (rr - s.rowsA))

    core_of = row // s.shard
    tile_of = (row % s.shard) // 128
    parity = colp & 1

    # order edges by (core, block, parity, tile)
    block_of = tile_of // s.BT
    key = ((core_of * s.nblocks + block_of) * 2 + parity) * s.tiles + tile_of
    order = np.argsort(key, kind="stable")
    row_s = row[order]
    colp_s = colp[order]
    core_s = core_of[order]
    tile_s = tile_of[order]
    par_s = parity[order]
    key_sorted = key[order]

    # per (core, tile, parity) counts; chunk counts = max over cores
    cnt_key = (core_s * s.tiles + tile_s) * 2 + par_s
    nkeys = n_cores * s.tiles * 2
    counts = np.bincount(cnt_key, minlength=nkeys).reshape(n_cores, s.tiles, 2)
    nch = _ceil_div(counts, 128).max(axis=0)  # [tiles, 2]
    s.nch = nch

    # chunk numbering: for block b: for parity p: for tile t in block
    choff = np.zeros((s.tiles, 2), dtype=np.int64)
    seg = []  # per (block, parity): (chunk_base, nchunks)
    a = 0
    for b in range(s.nblocks):
        t0, t1 = b * s.BT, min((b + 1) * s.BT, s.tiles)
        for p in range(2):
            base = a
            for t in range(t0, t1):
                choff[t, p] = a
                a += nch[t, p]
            seg.append((base, a - base))
    s.choff = choff
    s.totch = int(a)
    s.seg = seg
    s.maxseg = max(n for _, n in seg)

    # calls: per (block, parity) segment, balanced groups of <= G chunks
    G = 8
    s.G = G
    s.calls = []  # (chunk_base, nchunks, parity, block)
    chunk_call = np.zeros(s.totch, dtype=np.int64)
    chunk_col = np.zeros(s.totch, dtype=np.int64)
    s.calls_of_block = [[] for _ in range(s.nblocks)]
    si = 0
    for b in range(s.nblocks):
        for p in range(2):
            base, n = seg[si]
            si += 1
            if n == 0:
                continue
            ncalls = _ceil_div(n, G)
            sizes = [n // ncalls + (1 if i < n % ncalls else 0) for i in range(ncalls)]
            off = 0
            for sz in sizes:
                ci = len(s.calls)
                s.calls.append((base + off, sz, p, b))
                s.calls_of_block[b].append(ci)
                chunk_call[base + off:base + off + sz] = ci
                chunk_col[base + off:base + off + sz] = np.arange(sz)
                off += sz
    s.chunk_call = chunk_call
    s.chunk_col = chunk_col
    s.maxG = max(c[1] for c in s.calls)

    # per-core tables
    s.idx_tab = np.zeros((n_cores, 128, 8 * s.totch), dtype=np.int16)
    s.ld_tab = np.full((n_cores, 128, s.totch), -1.0, dtype=np.float32)
    for c in range(n_cores):
        for t in range(s.tiles):
            b = t // s.BT
            for p in range(2):
                n = int(nch[t, p])
                if n == 0:
                    continue
                k = ((c * s.nblocks + b) * 2 + p) * s.tiles + t
                lo = np.searchsorted(key_sorted, k, side="left")
                hi = np.searchsorted(key_sorted, k, side="right")
                cnt = hi - lo
                idx = np.zeros(n * 128, dtype=np.int64)
                idx[:cnt] = colp_s[lo:hi] >> 1
                ld = np.full(n * 128, -1.0, dtype=np.float32)
                ld[:cnt] = (row_s[lo:hi] - c * s.shard - t * 128).astype(np.float32)
                co = int(choff[t, p])
                # idx j -> [j%16, j//16], replicated across the 8 Q7 core groups
                wrapped = idx.astype(np.int16).reshape(-1, 16).T  # [16, n*8]
                s.idx_tab[c, :, 8 * co:8 * (co + n)] = np.tile(wrapped, (8, 1))
                s.ld_tab[c, :, co:co + n] = ld.reshape(n, 128).T

    # per-core dis table (partition = node % 128, col = tile), pad rows -> 0
    s.dis_t = np.zeros((n_cores, 128, s.tiles), dtype=np.float32)
    for c in range(n_cores):
        d = np.zeros(s.shard_pad, dtype=np.float32)
        d[:s.shard] = dis[c * s.shard:(c + 1) * s.shard]
        s.dis_t[c] = d.reshape(s.tiles, 128).T

    # pooling windows: split tiles into nw contiguous groups such that each
    # group's batch-id span is < 128 for every core
    batch = np.asarray(batch)
    s.B = int(batch.max()) + 1 if batch.size else 1
    for nw in range(1, s.tiles + 1):
        bounds = [round(i * s.tiles / nw) for i in range(nw + 1)]
        ok = True
        win_start = np.zeros((n_cores, nw), dtype=np.int64)
        for c in range(n_cores):
            for w in range(nw):
                n0 = c * s.shard + bounds[w] * 128
                n1 = min(c * s.shard + bounds[w + 1] * 128, (c + 1) * s.shard) - 1
                if n0 > n1:
                    win_start[c, w] = 0
                    continue
                b0, b1 = int(batch[n0]), int(batch[n1])
                if b1 - b0 > 127:
                    ok = False
                    break
                win_start[c, w] = b0
            if not ok:
                break
        if ok:
            s.nw = nw
            s.wbounds = bounds
            s.win_start = win_start
            break
    else:
        raise RuntimeError("no pooling window split found")
    s.win_of_tile = np.zeros(s.tiles, dtype=np.int64)
    for w in range(s.nw):
        s.win_of_tile[s.wbounds[w]:s.wbounds[w + 1]] = w

    # local graph ids per (core, tile): batch[node] - win_start, pad -> -1
    s.lg_tab = np.full((n_cores, 128, s.tiles), -1.0, dtype=np.float32)
    for c in range(n_cores):
        lg = np.full(s.shard_pad, -1.0, dtype=np.float32)
        bshard = batch[c * s.shard:(c + 1) * s.shard].astype(np.float32)
        for w in range(s.nw):
            t0, t1 = s.wbounds[w], s.wbounds[w + 1]
            n0, n1 = t0 * 128, min(t1 * 128, s.shard)
            lg[n0:n1] = bshard[n0:n1] - s.win_start[c, w]
        s.lg_tab[c] = lg.reshape(s.tiles, 128).T

    s.cnts = np.bincount(batch, minlength=s.B).astype(np.float32)
    return s


def _raw_gather(gp, out_ap, in_ap, idxs_ap, num_idxs, elem_size, elem_step,
                queue_num, sem=None):
    """dma_gather without the elem_size_bytes%256 wrapper assert.

    The ucode only requires the source row STRIDE to be a multiple of 256B
    (stride_bytes_256 field); the per-descriptor payload is elem_size bytes.
    128B fp8 payloads at 256B stride are legal and halve the DMA cost.
    """
    from concourse import mybir

    gp._assert_queue_num(queue_num)
    assert idxs_ap.dtype == mybir.dt.int16
    dts = mybir.dt.size(in_ap.dtype)
    stride_bytes = elem_step * dts
    assert stride_bytes % 256 == 0 and stride_bytes // 256 < 256
    assert in_ap.ap[0][0] == elem_step
    assert in_ap.ap[-1][1] == out_ap.ap[-1][1] == elem_size
    _in_ap = gp.lower_ap_dma(in_ap, for_custom_bir_dma=True)
    _idxs_ap = gp.lower_ap(idxs_ap)
    _out_ap = gp.lower_ap(out_ap)
    inst = gp.add_instruction(
        mybir.InstDMAGatherAnt(
            name=gp.bass.get_next_instruction_name(),
            ins=[*_in_ap, _idxs_ap, gp.lower_val_access(gp.to_reg(num_idxs))],
            outs=[_out_ap],
            transpose=False,
            num_idxs=num_idxs,
            elem_size=elem_size,
            stride_bytes_256=stride_bytes // 256,
            gen_mode=int(sem is not None),
            single_packet=True,
            queue_num=queue_num,
            sbuf_tokens_per_rank=0,
            sbuf_free_dim_per_rank=0,
            sbuf_free_dim_pad_per_rank=0,
            sbuf_byte_offset=0,
        )
    )
    if sem is not None:
        # prepare_only: prep retires at DGE-end; DMA fires via trigger_dma.
        # OnUpdate[0] must be the DMA completion sem (16 engine increments).
        inst.then_inc(sem, 16)
        return gp._track_prepare_only(inst, queue_num)
    return inst


def _build(s, layers=3):
    """Build the shared SPMD Bass/Tile program."""
    from contextlib import ExitStack

    import concourse.tile as tile
    from concourse import bacc, mybir

    DT = mybir.dt
    F32 = DT.float32
    BF16 = DT.bfloat16
    FP8 = DT.float8e4
    nc = bacc.Bacc("TRN2", target_bir_lowering=False, debug=False,
                   num_devices=s.n_cores, num_swdge_queues=4,
                   dynamic_dma_scratch_size=16384)

    g0 = nc.dram_tensor("g0", [s.npad, 128], FP8, kind="ExternalInput")
    w_in = nc.dram_tensor("w_in", [layers, 128, 128], BF16, kind="ExternalInput")
    b_in = nc.dram_tensor("b_in", [layers, 128, 128], F32, kind="ExternalInput")
    iota_in = nc.dram_tensor("iota_in", [128, 128], BF16, kind="ExternalInput")
    idx_in = nc.dram_tensor("idx_in", [128, 8 * s.totch], DT.int16, kind="ExternalInput")
    s_in = nc.dram_tensor("s_in", [128, s.totch * 128], FP8, kind="ExternalInput")
    dis_in = nc.dram_tensor("dis_in", [128, s.tiles], F32, kind="ExternalInput")
    lg_in = nc.dram_tensor("lg_in", [128, s.tiles], F32, kind="ExternalInput")
    pool_out = nc.dram_tensor("pool_out", [s.nw, 128, 128], F32, kind="ExternalOutput")

    g_bounceA = nc.dram_tensor("g_bounceA", [s.rowsA, 128], FP8)
    g_bounceB = nc.dram_tensor("g_bounceB", [s.rowsB, 128], FP8)
    g_full = [
        nc.dram_tensor(f"g_full{l}", [s.npad, 128], FP8, addr_space="Shared")
        for l in range(1, layers)
    ]

    relu = mybir.ActivationFunctionType.Relu
    copyf = mybir.ActivationFunctionType.Copy
    iseq = mybir.AluOpType.is_equal
    mult = mybir.AluOpType.mult
    add = mybir.AluOpType.add
    dr = mybir.MatmulPerfMode.DoubleRow

    with tile.TileContext(nc) as tc, ExitStack() as ctx:
        const = ctx.enter_context(tc.tile_pool(name="const", bufs=1))
        msgp = ctx.enter_context(tc.tile_pool(name="msg", bufs=36))
        sp = ctx.enter_context(tc.tile_pool(name="sp", bufs=4))
        pp = ctx.enter_context(tc.tile_pool(name="pp", bufs=4))
        zp = ctx.enter_context(tc.tile_pool(name="zp", bufs=8))
        houtp = ctx.enter_context(tc.tile_pool(name="hout", bufs=1))
        psz = ctx.enter_context(tc.tile_pool(name="psz", bufs=4, space="PSUM"))
        psu = ctx.enter_context(tc.tile_pool(name="psu", bufs=2, space="PSUM"))
        pspool = ctx.enter_context(tc.tile_pool(name="pspool", bufs=2, space="PSUM"))

        iota_t = const.tile([128, 128], BF16)
        nc.sync.dma_start(iota_t[:], iota_in[:])
        idx_t = const.tile([128, 8 * s.totch], DT.int16)
        nc.sync.dma_start(idx_t[:], idx_in[:])
        dis_t = const.tile([128, s.tiles], F32)
        nc.sync.dma_start(dis_t[:], dis_in[:])
        lg_t = const.tile([128, s.tiles], F32)
        nc.sync.dma_start(lg_t[:], lg_in[:])
        w_t = const.tile([128, layers, 128], BF16)
        nc.sync.dma_start(w_t[:], w_in.ap().rearrange("l p d -> p l d"))
        b_t = const.tile([128, layers, 128], F32)
        nc.sync.dma_start(b_t[:], b_in.ap().rearrange("l p d -> p l d"))

        hout = houtp.tile([128, s.tiles * 128], FP8)
        h3 = houtp.tile([128, s.tiles * 128], BF16)

        pool_pz = None

        for l in range(layers):
            g_src = g0 if l == 0 else g_full[l - 1]
            g_pair = g_src.ap().rearrange("(n two) d -> n (two d)", two=2)

            if l == layers - 1:
                pool_pz = [pspool.tile([128, 128], F32, tag="pool",
                                       name=f"poolpz{w}")
                           for w in range(s.nw)]

            call_tiles = {}
            seg_tiles = {}
            qn = [0]

            def issue_block(b):
                # stream S segments (host-precomputed one-hots) + gathers
                for p in range(2):
                    sbase, slen = s.seg[b * 2 + p]
                    if slen == 0:
                        continue
                    st = sp.tile([128, s.maxseg * 128], FP8, tag="S",
                                 name="st")
                    nc.sync.dma_start(
                        st[:, 0:slen * 128],
                        s_in[:, sbase * 128:(sbase + slen) * 128],
                    )
                    seg_tiles[(b, p)] = (st, sbase)
                for ci in s.calls_of_block[b]:
                    base, n, p, _ = s.calls[ci]
                    m = msgp.tile([128, s.maxG, 128], FP8, tag="msg",
                                  name="m")
                    _raw_gather(
                        nc.gpsimd,
                        m[:, 0:n, :],
                        g_pair[:, p * 128:(p + 1) * 128],
                        idx_t[:, 8 * base:8 * (base + n)],
                        n * 128,
                        128,
                        256,
                        qn[0],
                    )
                    qn[0] = (qn[0] + 1) % 4
                    call_tiles[ci] = m

            issue_block(0)
            for b in range(s.nblocks):
                if b + 1 < s.nblocks:
                    issue_block(b + 1)

                # process this block's tiles: one psum pass per tile
                t0, t1 = b * s.BT, min((b + 1) * s.BT, s.tiles)
                for t in range(t0, t1):
                    # gather (call, col, chunk, span) groups: DoubleRow pairs
                    groups = []
                    for p in range(2):
                        n = int(s.nch[t, p])
                        k = 0
                        while k < n:
                            gc = int(s.choff[t, p]) + k
                            ci = int(s.chunk_call[gc])
                            j = int(s.chunk_col[gc])
                            if k + 1 < n and int(s.chunk_call[gc + 1]) == ci:
                                groups.append((p, ci, j, gc, 2))
                                k += 2
                            else:
                                groups.append((p, ci, j, gc, 1))
                                k += 1
                    pz = psz.tile([128, 128], F32, tag="pz")
                    for i, (p, ci, j, gc, span) in enumerate(groups):
                        m = call_tiles[ci]
                        st, sbase = seg_tiles[(b, p)]
                        o = (gc - sbase) * 128
                        s_ap = st[:, o:o + span * 128] \
                            .rearrange("q (k d) -> q k d", k=span)
                        nc.tensor.matmul(
                            pz[:], m[:, j:j + span, :], s_ap,
                            start=(i == 0), stop=(i == len(groups) - 1),
                            perf_mode=(dr if span == 2 else None),
                        )
                    # epilogue: zT = copy(pz); pu = zT @ W; hout = relu(...)
                    zT = zp.tile([128, 128], BF16, tag="zT")
                    nc.scalar.activation(zT[:], pz[:], copyf)
                    pu = psu.tile([128, 128], F32, tag="pu")
                    nc.tensor.matmul(pu[:], zT[:], w_t[:, l, :], start=True, stop=True)
                    t1v = zp.tile([128, 128], F32, tag="t1")
                    nc.vector.tensor_scalar(t1v[:], pu[:], dis_t[:, t:t + 1], None, mult)
                    t2v = zp.tile([128, 128], F32, tag="t2")
                    nc.vector.tensor_tensor(t2v[:], t1v[:], b_t[:, l, :], add)
                    if l < layers - 1:
                        nc.scalar.activation(
                            hout[:, t * 128:(t + 1) * 128], t2v[:], relu,
                            scale=dis_t[:, t:t + 1],
                        )
                    else:
                        h3s = h3[:, t * 128:(t + 1) * 128]
                        nc.scalar.activation(h3s, t2v[:], relu)
                        # interleaved mean-pool accumulation
                        w = int(s.win_of_tile[t])
                        P = pp.tile([128, 128], BF16, tag="P")
                        nc.vector.tensor_scalar(
                            P[:], iota_t[:], lg_t[:, t:t + 1], None, iseq
                        )
                        nc.tensor.matmul(
                            pool_pz[w][:], P[:], h3s,
                            start=(t == s.wbounds[w]),
                            stop=(t == s.wbounds[w + 1] - 1),
                        )

            if l < layers - 1:
                nA = s.n_cores * s.rowsA
                nc.sync.dma_start(
                    g_bounceA.ap().rearrange("(t p) d -> p t d", p=128),
                    hout[:, 0:s.rowsA],
                )
                nc.gpsimd.collective_compute(
                    "AllGather",
                    mybir.AluOpType.bypass,
                    replica_groups=[list(range(s.n_cores))],
                    ins=[g_bounceA.ap().opt()],
                    outs=[g_full[l].ap()[0:nA].opt()],
                )
                nc.sync.dma_start(
                    g_bounceB.ap().rearrange("(t p) d -> p t d", p=128),
                    hout[:, s.rowsA:],
                )
                nc.gpsimd.collective_compute(
                    "AllGather",
                    mybir.AluOpType.bypass,
                    replica_groups=[list(range(s.n_cores))],
                    ins=[g_bounceB.ap().opt()],
                    outs=[g_full[l].ap()[nA:].opt()],
                )

        for w in range(s.nw):
            pres = zp.tile([128, 128], F32, tag="pres")
            nc.vector.tensor_copy(pres[:], pool_pz[w][:])
            nc.sync.dma_start(pool_out[w], pres[:])

    nc.compile()
    return nc


def _in_maps(x, Ws, bs, s):
    import ml_dtypes

    fp8 = ml_dtypes.float8_e4m3
    bf16 = ml_dtypes.bfloat16
    g0 = np.zeros((s.npad, 128), dtype=fp8)
    gx = s.dis[:, None] * np.asarray(x, dtype=np.float32)
    nA = s.n_cores * s.rowsA
    for c in range(s.n_cores):
        g0[c * s.rowsA:(c + 1) * s.rowsA] = \
            gx[c * s.shard:c * s.shard + s.rowsA].astype(fp8)
        nb = s.shard - s.rowsA
        g0[nA + c * s.rowsB:nA + c * s.rowsB + nb] = \
            gx[c * s.shard + s.rowsA:(c + 1) * s.shard].astype(fp8)

    iota = np.tile(np.arange(128, dtype=np.float32), (128, 1)).astype(bf16)
    wcat = np.stack([np.asarray(w, np.float32) for w in Ws]).astype(bf16)
    bcat = np.stack([np.tile(np.asarray(b, np.float32), (128, 1)) for b in bs])
    ar = np.arange(128, dtype=np.float32)

    in_maps = []
    for c in range(s.n_cores):
        S = (s.ld_tab[c][:, :, None] == ar).astype(fp8) \
            .reshape(128, s.totch * 128)
        in_maps.append({
            "g0": g0,
            "w_in": wcat,
            "b_in": bcat,
            "iota_in": iota,
            "idx_in": s.idx_tab[c],
            "s_in": S,
            "dis_in": s.dis_t[c],
            "lg_in": s.lg_tab[c],
        })

    return in_maps


def _run(x, edge_index, batch, Ws, bs, s, nc):
    from concourse.bass_utils import run_bass_kernel_spmd

    in_maps = _in_maps(x, Ws, bs, s)
    br = run_bass_kernel_spmd(nc, in_maps, list(range(s.n_cores)))

    acc = np.zeros((s.B + 128, 128), dtype=np.float32)
    for c in range(s.n_cores):
        po = np.asarray(br.results[c]["pool_out"]).reshape(s.nw, 128, 128)
        for w in range(s.nw):
            ws = int(s.win_start[c, w])
            acc[ws:ws + 128] += po[w]
    out = acc[:s.B] / np.maximum(s.cnts, 1.0)[:, None]
    return out, br


def kernel(x, edge_index, batch, W0, b0, W1, b1, W2, b2):
    x = np.asarray(x)
    edge_index = np.asarray(edge_index)
    batch = np.asarray(batch)
    s = _preprocess(x, edge_index, batch)
    nc = _build(s)
    out, _ = _run(x, edge_index, batch, [W0, W1, W2], [b0, b1, b2], s, nc)
    return out.astype(np.float32)
